# revision 1
# baseline (speedup 1.0000x reference)
"""Trainium2 Bass kernel for ragged-length attention (nn_Attention), 8-core SPMD.

Reference computation (per sample n, N=64, T=4096, D=128):
    energy[n, t] = <key[t, n, :], query[n, :]>
    mask[n, t]   = t < speech_len[n]
    score        = softmax(mask * energy, axis=t)   # multiplicative mask!
    context[n]   = sum_t score[n, t] * value[t, n, :]
    returns (context, mask)

Key observations exploited here:
  * softmax(mask*e) needs no max-subtraction for this data range, so
    s_t = exp(mask_t * e_t) can be computed tile-local and the single
    normalization by den = sum_t s_t can be folded out to the very end
    (done on host: it is N*D trivial scalar divisions).
  * For t >= speech_len, s_t == exp(0) == 1 exactly. So keys for t >= L are
    never needed (their energies are masked away), and the value rows there
    are weighted by a constant 1 -> matmul against a ones-vector, no exp.
  * Sharding: data-parallel over N (8 samples per core). Samples are
    assigned to (core, slot) by sorted length so that all 8 cores share one
    SPMD graph whose per-slot key-prefix length C[slot] is tight.

Device layout per core (slot n in 0..7, C = C[n] key tiles of 128):
    qT    (128, 8)     f32   qT[d, n] = query[sample(n), d]
    keyT  (8, 128, 4096)     keyT[n, d, t] = key[t, sample(n), d]
    val   (8, 128, 4096)     val[n, p, 128c+v] = value[128c+p, sample(n), v]
    maskR (128, 256)         maskR[p, 32n+c] = (128c+p) < L_n
outputs:
    ctxT  (128, 8)     f32   unnormalized context.T
    part  (128, 8)     f32   per-partition sums of s over the exp region
    maskO (128, 256)   f32   mask pass-through
"""
import numpy as np

N, T, D = 64, 4096, 128
NC = 8          # cores
NL = 8          # samples (slots) per core
CT = T // 128   # 32 column tiles of 128 t-steps

TRACE = False
LAST_EXEC_NS = None

_cache: dict = {}


def _build(Cs: tuple) -> "object":
    """Build + compile the SPMD Bass graph for per-slot key-tile counts Cs."""
    import concourse.tile as tile
    from concourse import bacc, mybir

    f32 = mybir.dt.float32
    EXP = mybir.ActivationFunctionType.Exp

    nc = bacc.Bacc("TRN2", target_bir_lowering=False, debug=False, num_devices=NC)
    qT_d = nc.dram_tensor("qT", [128, NL], f32, kind="ExternalInput").ap()
    keyT_d = nc.dram_tensor("keyT", [NL, 128, T], f32, kind="ExternalInput").ap()
    val_d = nc.dram_tensor("val", [NL, 128, T], f32, kind="ExternalInput").ap()
    maskR_d = nc.dram_tensor("maskR", [128, NL * CT], f32, kind="ExternalInput").ap()
    ctxT_d = nc.dram_tensor("ctxT", [128, NL], f32, kind="ExternalOutput").ap()
    part_d = nc.dram_tensor("part", [128, NL], f32, kind="ExternalOutput").ap()
    maskO_d = nc.dram_tensor("maskO", [128, NL * CT], f32, kind="ExternalOutput").ap()

    with tile.TileContext(nc) as tc:
        with (
            tc.tile_pool(name="key", bufs=3) as kpool,
            tc.tile_pool(name="val", bufs=3) as vpool,
            tc.tile_pool(name="s", bufs=3) as spool,
            tc.tile_pool(name="misc", bufs=1) as mpool,
            tc.tile_pool(name="pe", bufs=3, space="PSUM") as pepool,
            tc.tile_pool(name="pctx", bufs=1, space="PSUM") as pcpool,
        ):
            qt_sb = mpool.tile([128, NL], f32)
            nc.sync.dma_start(qt_sb[:], qT_d[:])
            mask_sb = mpool.tile([128, NL * CT], f32)
            nc.sync.dma_start(mask_sb[:], maskR_d[:])
            ones_sb = mpool.tile([128, 1], f32)
            nc.vector.memset(ones_sb[:], 1.0)
            part_sb = mpool.tile([128, NL], f32)
            nc.vector.memset(part_sb[:], 0.0)
            psum_ctx = pcpool.tile([128, NL], f32)

            staged = {}

            def load_and_energy(n):
                C = Cs[n]
                s = None
                if C > 0:
                    kt = kpool.tile([128, C * 128], f32, tag="key")
                    nc.sync.dma_start(kt[:], keyT_d[n, :, 0 : C * 128])
                vt = vpool.tile([128, T], f32, tag="val")
                nc.sync.dma_start(vt[:], val_d[n])
                if C > 0:
                    pe = pepool.tile([128, C], f32, tag="pe")
                    for c in range(C):
                        nc.tensor.matmul(
                            pe[:, c : c + 1],
                            kt[:, c * 128 : (c + 1) * 128],
                            qt_sb[:, n : n + 1],
                            start=True,
                            stop=True,
                        )
                    masked = spool.tile([128, C], f32, tag="masked")
                    nc.vector.tensor_mul(
                        masked[:], pe[:, 0:C], mask_sb[:, n * CT : n * CT + C]
                    )
                    s = spool.tile([128, C], f32, tag="s")
                    nc.scalar.activation(
                        s[:], masked[:], EXP, accum_out=part_sb[:, n : n + 1]
                    )
                staged[n] = (vt, s)

            def context_mms(n):
                vt, s = staged.pop(n)
                C = Cs[n]
                for c in range(CT):
                    rhs = s[:, c : c + 1] if c < C else ones_sb[:, 0:1]
                    nc.tensor.matmul(
                        psum_ctx[:, n : n + 1],
                        vt[:, c * 128 : (c + 1) * 128],
                        rhs,
                        start=(c == 0),
                        stop=(c == CT - 1),
                    )

            for n in range(NL):
                load_and_energy(n)
                context_mms(n)

            ctx_sb = mpool.tile([128, NL], f32)
            nc.vector.tensor_copy(ctx_sb[:], psum_ctx[:])
            nc.sync.dma_start(ctxT_d[:], ctx_sb[:])
            nc.sync.dma_start(part_d[:], part_sb[:])
            nc.sync.dma_start(maskO_d[:], mask_sb[:])

    nc.compile()
    return nc


def kernel(query, key, value, speech_len):
    global LAST_EXEC_NS
    from concourse.bass_utils import run_bass_kernel_spmd

    query = np.asarray(query, dtype=np.float32)
    key = np.asarray(key, dtype=np.float32)
    value = np.asarray(value, dtype=np.float32)
    sl_in = np.asarray(speech_len)
    L = sl_in.astype(np.int64).clip(0, T)

    # (core, slot) assignment: sort by length; slot n holds sorted ranks
    # [8n, 8n+8), one per core -> per-slot max length is tight and all cores
    # share one SPMD graph.
    order = np.argsort(L, kind="stable")
    perm = order.reshape(NL, NC)  # perm[n, i] = sample for core i, slot n
    Cs = tuple(
        int(-(-int(L[perm[n]].max()) // 128)) for n in range(NL)
    )  # ceil(max L in slot / 128)

    if Cs not in _cache:
        _cache[Cs] = _build(Cs)
    nc = _cache[Cs]

    t_idx = np.arange(T)
    in_maps = []
    for i in range(NC):
        idx = perm[:, i]  # 8 sample indices for this core
        qT = np.ascontiguousarray(query[idx, :].T)  # (128, 8)
        keyT = np.ascontiguousarray(
            key[:, idx, :].transpose(1, 2, 0)
        )  # (8, 128, 4096)
        val = np.ascontiguousarray(
            value[:, idx, :].reshape(CT, 128, NL, D).transpose(2, 1, 0, 3)
        ).reshape(NL, 128, T)
        m = (t_idx[None, :] < L[idx][:, None]).astype(np.float32)  # (8, 4096)
        maskR = np.ascontiguousarray(
            m.reshape(NL, CT, 128).transpose(2, 0, 1)
        ).reshape(128, NL * CT)
        in_maps.append({"qT": qT, "keyT": keyT, "val": val, "maskR": maskR})

    res = run_bass_kernel_spmd(nc, in_maps, core_ids=list(range(NC)), trace=TRACE)
    LAST_EXEC_NS = res.exec_time_ns

    context = np.zeros((N, D), dtype=np.float32)
    mask = np.zeros((N, T), dtype=np.float32)
    for i in range(NC):
        idx = perm[:, i]
        ctxT = res.results[i]["ctxT"]  # (128, 8)
        part = res.results[i]["part"]  # (128, 8)
        maskO = res.results[i]["maskO"]  # (128, 256)
        den = part.sum(axis=0, dtype=np.float64) + (T - 128 * np.asarray(Cs))
        context[idx, :] = (ctxT.astype(np.float64) / den[None, :]).T.astype(
            np.float32
        )
        mask[idx, :] = (
            maskO.reshape(128, NL, CT).transpose(1, 2, 0).reshape(NL, T)
        )
    return context, mask


# revision 2
# speedup vs baseline: 3.5711x; 3.5711x over previous
"""Trainium2 Bass kernel for ragged-length attention (nn_Attention), 8-core SPMD.

Reference computation (per sample n, N=64, T=4096, D=128):
    energy[n, t] = <key[t, n, :], query[n, :]>
    mask[n, t]   = t < speech_len[n]
    score        = softmax(mask * energy, axis=t)   # multiplicative mask!
    context[n]   = sum_t score[n, t] * value[t, n, :]
    returns (context, mask)

Design:
  * softmax(mask*e) needs no max-subtraction for this data range, so
    s_t = exp(mask_t * e_t) is computed tile-local; the normalization by
    den = sum_t s_t folds out to a trivial host-side scalar division.
  * For t >= speech_len, s_t == exp(0) == 1 exactly: keys there are never
    read, and the value tail is contracted against a ones-vector with wide
    (512-col) moving matmuls.
  * Sharding: data-parallel over N (8 samples/core), samples assigned to
    (core, slot) by sorted length so all 8 cores share one SPMD graph with
    tight per-slot key-prefix length C[slot].
  * Matmul operands are bf16 (f32 matmul runs 2-pass LOW_HIGH on PE);
    accumulation stays f32 in PSUM.

Device layout per core (slot n in 0..7, C = C[n] key tiles of 128):
    qT    (128, 8)     bf16  qT[d, n] = query[sample(n), d]
    keyT  (8, 128, 4096) bf16 keyT[n, d, t] = key[t, sample(n), d]
    val   (8, 128, 4096) bf16 val[n, p, 128c+v] = value[128c+p, sample(n), v]
    maskR (128, 256)   f32   maskR[p, 32n+c] = (128c+p) < L_n
outputs:
    ctxT  (128, 8)     f32   unnormalized context.T from the exp(s) region
    part  (128, 8)     f32   per-partition sums of s over the exp region
    tail  (1, 4096)    f32   per-slot folded value tail sums (4 groups of 128)
    maskO (128, 256)   f32   mask pass-through
"""
import numpy as np
import ml_dtypes

N, T, D = 64, 4096, 128
NC = 8          # cores
NL = 8          # samples (slots) per core
CT = T // 128   # 32 column tiles of 128 t-steps

TRACE = False
LAST_EXEC_NS = None

_cache: dict = {}


def _build(Cs: tuple) -> "object":
    """Build + compile the SPMD Bass graph for per-slot key-tile counts Cs."""
    import concourse.tile as tile
    from concourse import bacc, mybir

    f32 = mybir.dt.float32
    bf16 = mybir.dt.bfloat16
    EXP = mybir.ActivationFunctionType.Exp

    nc = bacc.Bacc("TRN2", target_bir_lowering=False, debug=False, num_devices=NC)
    qT_d = nc.dram_tensor("qT", [128, NL], bf16, kind="ExternalInput").ap()
    keyT_d = nc.dram_tensor("keyT", [NL, 128, T], bf16, kind="ExternalInput").ap()
    val_d = nc.dram_tensor("val", [NL, 128, T], bf16, kind="ExternalInput").ap()
    maskR_d = nc.dram_tensor("maskR", [128, NL * CT], f32, kind="ExternalInput").ap()
    ctxT_d = nc.dram_tensor("ctxT", [128, NL], f32, kind="ExternalOutput").ap()
    part_d = nc.dram_tensor("part", [128, NL], f32, kind="ExternalOutput").ap()
    tail_d = nc.dram_tensor("tail", [1, NL * 512], f32, kind="ExternalOutput").ap()
    maskO_d = nc.dram_tensor("maskO", [128, NL * CT], f32, kind="ExternalOutput").ap()

    with tile.TileContext(nc) as tc:
        with (
            tc.tile_pool(name="key", bufs=3) as kpool,
            tc.tile_pool(name="val", bufs=4) as vpool,
            tc.tile_pool(name="s", bufs=3) as spool,
            tc.tile_pool(name="misc", bufs=1) as mpool,
            tc.tile_pool(name="pe", bufs=3, space="PSUM") as pepool,
            tc.tile_pool(name="pctx", bufs=1, space="PSUM") as pcpool,
            tc.tile_pool(name="ptail", bufs=2, space="PSUM") as ptpool,
        ):
            qt_sb = mpool.tile([128, NL], bf16)
            nc.sync.dma_start(qt_sb[:], qT_d[:])
            mask_sb = mpool.tile([128, NL * CT], f32)
            nc.sync.dma_start(mask_sb[:], maskR_d[:])
            ones_sb = mpool.tile([128, 1], bf16)
            nc.vector.memset(ones_sb[:], 1.0)
            part_sb = mpool.tile([128, NL], f32)
            nc.vector.memset(part_sb[:], 0.0)
            tail_sb = mpool.tile([1, NL * 512], f32)
            nc.vector.memset(tail_sb[:], 0.0)
            psum_ctx = pcpool.tile([128, NL], f32)

            staged = {}

            def load_and_energy(n):
                C = Cs[n]
                s = None
                if C > 0:
                    kt = kpool.tile([128, C * 128], bf16, tag="key")
                    nc.sync.dma_start(kt[:], keyT_d[n, :, 0 : C * 128])
                vt = vpool.tile([128, T], bf16, tag="val")
                nc.sync.dma_start(vt[:], val_d[n])
                if C > 0:
                    pe = pepool.tile([128, C], f32, tag="pe")
                    for c in range(C):
                        nc.tensor.matmul(
                            pe[:, c : c + 1],
                            kt[:, c * 128 : (c + 1) * 128],
                            qt_sb[:, n : n + 1],
                            start=True,
                            stop=True,
                        )
                    masked = spool.tile([128, C], f32, tag="masked")
                    nc.vector.tensor_mul(
                        masked[:], pe[:, 0:C], mask_sb[:, n * CT : n * CT + C]
                    )
                    s = spool.tile([128, C], bf16, tag="s")
                    nc.scalar.activation(
                        s[:], masked[:], EXP, accum_out=part_sb[:, n : n + 1]
                    )
                staged[n] = (vt, s)

            def context_mms(n):
                vt, s = staged.pop(n)
                C = Cs[n]
                # exp-weighted region: per-tile matmuls, value tile stationary
                for c in range(C):
                    nc.tensor.matmul(
                        psum_ctx[:, n : n + 1],
                        vt[:, c * 128 : (c + 1) * 128],
                        s[:, c : c + 1],
                        start=(c == 0),
                        stop=(c == C - 1),
                    )
                # ones-weighted tail: wide moving matmuls, ones stationary
                ltail = (CT - C) * 128
                if ltail > 0:
                    pt = ptpool.tile([1, 512], f32, tag="pt")
                    nmm = -(-ltail // 512)
                    for m in range(nmm):
                        x0 = C * 128 + 512 * m
                        w = min(512, T - x0)
                        nc.tensor.matmul(
                            pt[0:1, 0:w],
                            ones_sb[:],
                            vt[:, x0 : x0 + w],
                            start=(m == 0),
                            stop=(m == nmm - 1),
                        )
                    wv = min(512, ltail)
                    nc.vector.tensor_copy(
                        tail_sb[0:1, n * 512 : n * 512 + wv], pt[0:1, 0:wv]
                    )

            for n in range(NL):
                load_and_energy(n)
                if n >= 1:
                    context_mms(n - 1)
            context_mms(NL - 1)

            ctx_sb = mpool.tile([128, NL], f32)
            nc.vector.tensor_copy(ctx_sb[:], psum_ctx[:])
            nc.sync.dma_start(ctxT_d[:], ctx_sb[:])
            nc.sync.dma_start(part_d[:], part_sb[:])
            nc.sync.dma_start(tail_d[:], tail_sb[:])
            nc.sync.dma_start(maskO_d[:], mask_sb[:])

    nc.compile()
    return nc


def kernel(query, key, value, speech_len):
    global LAST_EXEC_NS
    from concourse.bass_utils import run_bass_kernel_spmd

    query = np.asarray(query, dtype=np.float32)
    key = np.asarray(key, dtype=np.float32)
    value = np.asarray(value, dtype=np.float32)
    sl_in = np.asarray(speech_len)
    L = sl_in.astype(np.int64).clip(0, T)

    # (core, slot) assignment: sort by length; slot n holds sorted ranks
    # [8n, 8n+8), one per core -> per-slot max length is tight and all cores
    # share one SPMD graph.
    order = np.argsort(L, kind="stable")
    perm = order.reshape(NL, NC)  # perm[n, i] = sample for core i, slot n
    Cs = tuple(int(-(-int(L[perm[n]].max()) // 128)) for n in range(NL))

    if Cs not in _cache:
        _cache[Cs] = _build(Cs)
    nc = _cache[Cs]

    bf16 = ml_dtypes.bfloat16
    t_idx = np.arange(T)
    in_maps = []
    for i in range(NC):
        idx = perm[:, i]  # 8 sample indices for this core
        qT = np.ascontiguousarray(query[idx, :].T).astype(bf16)  # (128, 8)
        keyT = np.ascontiguousarray(
            key[:, idx, :].transpose(1, 2, 0)
        ).astype(bf16)  # (8, 128, 4096)
        val = (
            np.ascontiguousarray(
                value[:, idx, :].reshape(CT, 128, NL, D).transpose(2, 1, 0, 3)
            )
            .reshape(NL, 128, T)
            .astype(bf16)
        )
        m = (t_idx[None, :] < L[idx][:, None]).astype(np.float32)  # (8, 4096)
        maskR = np.ascontiguousarray(
            m.reshape(NL, CT, 128).transpose(2, 0, 1)
        ).reshape(128, NL * CT)
        in_maps.append({"qT": qT, "keyT": keyT, "val": val, "maskR": maskR})

    res = run_bass_kernel_spmd(nc, in_maps, core_ids=list(range(NC)), trace=TRACE)
    LAST_EXEC_NS = res.exec_time_ns

    context = np.zeros((N, D), dtype=np.float32)
    mask = np.zeros((N, T), dtype=np.float32)
    for i in range(NC):
        idx = perm[:, i]
        ctxT = res.results[i]["ctxT"].astype(np.float64)  # (128, 8)
        part = res.results[i]["part"]  # (128, 8)
        tail = res.results[i]["tail"].reshape(NL, 4, 128).astype(np.float64)
        maskO = res.results[i]["maskO"]  # (128, 256)
        for n in range(NL):
            C = Cs[n]
            den = float(part[:, n].sum(dtype=np.float64)) + (T - 128 * C)
            acc = np.zeros(D, dtype=np.float64)
            if C > 0:
                acc += ctxT[:, n]
            ngroups = min(4, CT - C)
            for g in range(ngroups):
                acc += tail[n, g]
            context[idx[n], :] = (acc / den).astype(np.float32)
        mask[idx, :] = (
            maskO.reshape(128, NL, CT).transpose(1, 2, 0).reshape(NL, T)
        )
    return context, mask
